# revision 17
# baseline (speedup 1.0000x reference)
"""Sparse L1-distance attention (nn_L1AttnSparse) on 8 Trainium2 NeuronCores.

Sharding: dst tokens split across 8 cores (256 each). Per (batch, chunk of
128 dst): gather the 32 k rows per dst via SWDGE dma_gather (bf16, w-major
rows so DVE broadcasts keep the last dim packed), compute L1 scores with
scalar_tensor_tensor ops (4x DVE mode) + an in-place fold tree (abs fused
via abs_max), softmax over slots, then gather v rows with the same indices,
multiply by weights and fold over slots.  All heavy elementwise work runs
in bf16 with packed last dims; reductions tail in fp32 for precision.
"""

import sys

sys.path.insert(0, "/opt/trn_rl_repo")

import numpy as np
from ml_dtypes import bfloat16

import concourse.bass as bass
import concourse.tile as tile
from concourse import bacc, mybir
from concourse.bass_utils import run_bass_kernel_spmd

BS = 2
N_TOK = 2048
NH = 8
W = 64
S = 32  # dst_mxlen
HW = NH * W  # 512 values per (b, tok) row
N_CORES = 8
DT = N_TOK // N_CORES  # dst tokens per core = 256
CHUNKS = DT // 128  # dst chunks of 128 per core = 2
EDGES = 128 * S  # edges per (b, chunk) = 4096
SCALE = 1.0 / np.sqrt(W)


def _wrap_idx(flat):
    """int16 index list -> [128, n/16] tile layout: idx i at [i%16, i//16],
    replicated down the 8 groups of 16 partitions."""
    n = flat.shape[0]
    w16 = np.zeros((16, n // 16), dtype=np.int16)
    w16[np.arange(n) % 16, np.arange(n) // 16] = flat
    return np.tile(w16, (8, 1))


def build_kernel():
    nc = bacc.Bacc(
        "TRN2", target_bir_lowering=False, debug=False, num_devices=N_CORES,
        dynamic_dma_scratch_size=16384 * 4,
    )
    f32 = mybir.dt.float32
    bf16 = mybir.dt.bfloat16
    i16 = mybir.dt.int16
    AL = mybir.AluOpType

    kf = nc.dram_tensor("kf", [BS * N_TOK, HW], bf16, kind="ExternalInput").ap()
    vf = nc.dram_tensor("vf", [BS * N_TOK, HW], bf16, kind="ExternalInput").ap()
    qc = nc.dram_tensor("qc", [BS, CHUNKS, 128, HW], bf16, kind="ExternalInput").ap()
    idx = nc.dram_tensor(
        "idx", [BS, CHUNKS, 4, 128, EDGES // 64], i16, kind="ExternalInput"
    ).ap()
    oc = nc.dram_tensor("oc", [BS, CHUNKS, 128, HW], bf16, kind="ExternalOutput").ap()

    with tile.TileContext(nc) as tc:
        with (
            tc.tile_pool(name="kgp", bufs=2) as kgp,
            tc.tile_pool(name="vgp", bufs=2) as vgp,
            tc.tile_pool(name="qp", bufs=2) as qp,
            tc.tile_pool(name="idxp", bufs=2) as idxp,
            tc.tile_pool(name="smp", bufs=2) as smp,
        ):
            for b in range(BS):
                for c in range(CHUNKS):
                    q_t = qp.tile([128, HW], bf16, tag="q")
                    nc.sync.dma_start(out=q_t[:], in_=qc[b, c])
                    its = []
                    for hf in range(4):
                        it = idxp.tile([128, EDGES // 64], i16, tag=f"idx{hf}")
                        nc.sync.dma_start(out=it[:], in_=idx[b, c, hf])
                        its.append(it)

                    kg = kgp.tile([128, S, HW], bf16, tag="kg")
                    vg = vgp.tile([128, S, HW], bf16, tag="vg")
                    for hf in range(4):
                        nc.gpsimd.dma_gather(
                            kg[:, 8 * hf : 8 * hf + 8, :], kf, its[hf][:],
                            EDGES // 4, EDGES // 4, HW, queue_num=0,
                        )
                    for hf in range(4):
                        nc.gpsimd.dma_gather(
                            vg[:, 8 * hf : 8 * hf + 8, :], vf, its[hf][:],
                            EDGES // 4, EDGES // 4, HW, queue_num=0,
                        )

                    # --- scores: L[d, s, h] = sum_w |k - q| (w-major rows,
                    # so w-folds are contiguous-half folds: 3D STT at 4x) ---
                    kgv = kg[:]  # [128, S, HW], HW = (w major, h minor)
                    qv = q_t[:, None, :].to_broadcast([128, S, HW])
                    # kg <- kg - q  (in place, 4x mode)
                    nc.vector.scalar_tensor_tensor(
                        out=kgv, in0=kgv, scalar=0.0, in1=qv,
                        op0=AL.bypass, op1=AL.subtract,
                    )
                    # abs in place: kg <- max(-kg, kg)
                    nc.vector.scalar_tensor_tensor(
                        out=kgv, in0=kgv, scalar=-1.0, in1=kgv,
                        op0=AL.mult, op1=AL.max,
                    )
                    # w-folds 64 -> 2 (bf16 partials stay small)
                    for wd in (256, 128, 64, 32, 16):
                        nc.vector.scalar_tensor_tensor(
                            out=kgv[:, :, 0:wd],
                            in0=kgv[:, :, 0:wd],
                            scalar=0.0,
                            in1=kgv[:, :, wd : 2 * wd],
                            op0=AL.bypass, op1=AL.add,
                        )
                    # fp32 tail: L[p, s, h] = lo8 + hi8
                    L = smp.tile([128, S * NH], f32, tag="L")
                    nc.vector.tensor_tensor(
                        out=L[:].rearrange("p (s h) -> p s h", h=NH),
                        in0=kg[:, :, 0:8],
                        in1=kg[:, :, 8:16],
                        op=AL.add,
                    )
                    # --- softmax over s (per head); shift-free: exp args are
                    # in [-14, -5], safe in fp32 ---
                    E = L
                    nc.scalar.activation(
                        out=E[:], in_=L[:], func=mybir.ActivationFunctionType.Exp,
                        scale=-SCALE,
                    )
                    den = smp.tile([128, NH], f32, tag="den")
                    nc.vector.tensor_reduce(
                        out=den[:],
                        in_=E[:].rearrange("p (s h) -> p h s", h=NH),
                        axis=mybir.AxisListType.X,
                        op=AL.add,
                    )
                    rden = smp.tile([128, NH], f32, tag="rden")
                    nc.vector.reciprocal(rden[:], den[:])
                    Wt = smp.tile([128, S * NH], bf16, tag="Wt")
                    nc.vector.tensor_tensor(
                        out=Wt[:].rearrange("p (s h) -> p s h", h=NH),
                        in0=E[:].rearrange("p (s h) -> p s h", h=NH),
                        in1=rden[:, None, :].to_broadcast([128, S, NH]),
                        op=AL.mult,
                    )
                    # --- weighted v sum: vg <- vg * Wt, fold s 32 -> 1 ---
                    vgv = vg[:].rearrange("p s (w h) -> p s w h", h=NH)
                    wv = Wt[:].rearrange("p (s h) -> p s h", h=NH)[
                        :, :, None, :
                    ].to_broadcast([128, S, W, NH])
                    nc.vector.tensor_tensor(
                        out=vgv, in0=vgv, in1=wv, op=AL.mult,
                    )
                    for sd in (16, 8, 4, 2, 1):
                        nc.vector.scalar_tensor_tensor(
                            out=vg[:, 0:sd, :],
                            in0=vg[:, 0:sd, :],
                            scalar=0.0,
                            in1=vg[:, sd : 2 * sd, :],
                            op0=AL.bypass, op1=AL.add,
                        )
                    nc.sync.dma_start(out=oc[b, c], in_=vg[:, 0, :])
    nc.compile()
    return nc


_NC_CACHE = None


def kernel(v, q, k, coo, dst_mxlen):
    global _NC_CACHE
    assert int(dst_mxlen) == S
    v = np.asarray(v, dtype=np.float32)
    q = np.asarray(q, dtype=np.float32)
    k = np.asarray(k, dtype=np.float32)
    coo = np.asarray(coo)

    # src table: srct[t, s] = src index of edge (dst=t, slot=s)
    srct = np.zeros((N_TOK, S), dtype=np.int64)
    srct[coo[:, 0], coo[:, 2]] = coo[:, 1]

    # w-major bf16 tables: row (b, tok) = [w, h] flattened
    kf = np.ascontiguousarray(
        k.transpose(0, 1, 3, 2).reshape(BS * N_TOK, HW)
    ).astype(bfloat16)
    vf = np.ascontiguousarray(
        v.transpose(0, 1, 3, 2).reshape(BS * N_TOK, HW)
    ).astype(bfloat16)

    if _NC_CACHE is None:
        _NC_CACHE = build_kernel()
    nc = _NC_CACHE

    in_maps = []
    for core in range(N_CORES):
        lo = core * DT
        qcore = np.ascontiguousarray(
            q[:, lo : lo + DT].transpose(0, 1, 3, 2).reshape(BS, CHUNKS, 128, HW)
        ).astype(bfloat16)
        idx = np.zeros((BS, CHUNKS, 4, 128, EDGES // 64), dtype=np.int16)
        for b in range(BS):
            for c in range(CHUNKS):
                # edge i = s*128 + p  ->  row b*2048 + srct[lo + c*128 + p, s]
                flat = (
                    b * N_TOK + srct[lo + c * 128 : lo + (c + 1) * 128, :].T
                ).reshape(-1).astype(np.int16)
                for hf in range(4):
                    idx[b, c, hf] = _wrap_idx(
                        flat[EDGES // 4 * hf : EDGES // 4 * (hf + 1)]
                    )
        in_maps.append({"kf": kf, "vf": vf, "qc": qcore, "idx": idx})

    res = run_bass_kernel_spmd(nc, in_maps, list(range(N_CORES)))
    out = np.empty((BS, N_TOK, NH, W), dtype=np.float32)
    for core in range(N_CORES):
        lo = core * DT
        o = np.asarray(res.results[core]["oc"]).astype(np.float32)
        o = o.reshape(BS, CHUNKS, 128, W, NH).transpose(0, 1, 2, 4, 3)
        out[:, lo : lo + DT] = o.reshape(BS, DT, NH, W)
    return out


# revision 18
# speedup vs baseline: 1.7514x; 1.7514x over previous
"""Sparse L1-distance attention (nn_L1AttnSparse) on 8 Trainium2 NeuronCores.

Sharding: dst tokens split across 8 cores (256 each). Per (batch, chunk of
128 dst): gather the 32 k rows per dst via SWDGE dma_gather (bf16, w-major
rows so DVE broadcasts keep the last dim packed), compute L1 scores with
scalar_tensor_tensor ops (4x DVE mode) + an in-place fold tree (abs fused
via abs_max), softmax over slots, then gather v rows with the same indices,
multiply by weights and fold over slots.  All heavy elementwise work runs
in bf16 with packed last dims; reductions tail in fp32 for precision.
"""

import sys

sys.path.insert(0, "/opt/trn_rl_repo")

import numpy as np
from ml_dtypes import bfloat16

import concourse.bass as bass
import concourse.tile as tile
from concourse import bacc, mybir
from concourse.bass_utils import run_bass_kernel_spmd

BS = 2
N_TOK = 2048
NH = 8
W = 64
S = 32  # dst_mxlen
HW = NH * W  # 512 values per (b, tok) row
N_CORES = 8
DT = N_TOK // N_CORES  # dst tokens per core = 256
CHUNKS = DT // 128  # dst chunks of 128 per core = 2
EDGES = 128 * S  # edges per (b, chunk) = 4096
SCALE = 1.0 / np.sqrt(W)


def _wrap_idx(flat):
    """int16 index list -> [128, n/16] tile layout: idx i at [i%16, i//16],
    replicated down the 8 groups of 16 partitions."""
    n = flat.shape[0]
    w16 = np.zeros((16, n // 16), dtype=np.int16)
    w16[np.arange(n) % 16, np.arange(n) // 16] = flat
    return np.tile(w16, (8, 1))


def build_kernel():
    nc = bacc.Bacc(
        "TRN2", target_bir_lowering=False, debug=False, num_devices=N_CORES,
        dynamic_dma_scratch_size=16384 * 4,
    )
    f32 = mybir.dt.float32
    bf16 = mybir.dt.bfloat16
    i16 = mybir.dt.int16
    AL = mybir.AluOpType

    kf = nc.dram_tensor("kf", [BS * N_TOK, HW], bf16, kind="ExternalInput").ap()
    vf = nc.dram_tensor("vf", [BS * N_TOK, HW], bf16, kind="ExternalInput").ap()
    qc = nc.dram_tensor("qc", [BS, CHUNKS, 128, HW], bf16, kind="ExternalInput").ap()
    idx = nc.dram_tensor(
        "idx", [BS, CHUNKS, 4, 128, EDGES // 64], i16, kind="ExternalInput"
    ).ap()
    oc = nc.dram_tensor("oc", [BS, CHUNKS, 128, HW], bf16, kind="ExternalOutput").ap()

    with tile.TileContext(nc) as tc:
        with (
            tc.tile_pool(name="kgp", bufs=2) as kgp,
            tc.tile_pool(name="vgp", bufs=2) as vgp,
            tc.tile_pool(name="qp", bufs=2) as qp,
            tc.tile_pool(name="idxp", bufs=2) as idxp,
            tc.tile_pool(name="smp", bufs=2) as smp,
        ):
            for b in range(BS):
                for c in range(CHUNKS):
                    q_t = qp.tile([128, HW], bf16, tag="q")
                    nc.sync.dma_start(out=q_t[:], in_=qc[b, c])
                    its = []
                    for hf in range(4):
                        it = idxp.tile([128, EDGES // 64], i16, tag=f"idx{hf}")
                        nc.sync.dma_start(out=it[:], in_=idx[b, c, hf])
                        its.append(it)

                    kg = kgp.tile([128, S, HW], bf16, tag="kg")
                    vg = vgp.tile([128, S, HW], bf16, tag="vg")
                    for hf in range(4):
                        nc.gpsimd.dma_gather(
                            kg[:, 8 * hf : 8 * hf + 8, :], kf, its[hf][:],
                            EDGES // 4, EDGES // 4, HW, queue_num=0,
                        )
                    for hf in range(4):
                        nc.gpsimd.dma_gather(
                            vg[:, 8 * hf : 8 * hf + 8, :], vf, its[hf][:],
                            EDGES // 4, EDGES // 4, HW, queue_num=0,
                        )

                    # --- scores: L[d, s, h] = sum_w |k - q| (w-major rows,
                    # so w-folds are contiguous-half folds: 3D STT at 4x) ---
                    kgv = kg[:]  # [128, S, HW], HW = (w major, h minor)
                    qv = q_t[:, None, :].to_broadcast([128, S, HW])
                    # kg <- kg - q  (in place, TT bf16 2x)
                    nc.vector.tensor_tensor(
                        out=kgv, in0=kgv, in1=qv, op=AL.subtract,
                    )
                    # abs on the idle ACT engine
                    nc.scalar.activation(
                        out=kgv, in_=kgv,
                        func=mybir.ActivationFunctionType.Abs,
                    )
                    # w-folds 64 -> 2 (bf16 partials stay small)
                    for wd in (256, 128, 64, 32, 16):
                        nc.vector.tensor_tensor(
                            out=kgv[:, :, 0:wd],
                            in0=kgv[:, :, 0:wd],
                            in1=kgv[:, :, wd : 2 * wd],
                            op=AL.add,
                        )
                    # fp32 tail: L[p, s, h] = lo8 + hi8
                    L = smp.tile([128, S * NH], f32, tag="L")
                    nc.vector.tensor_tensor(
                        out=L[:].rearrange("p (s h) -> p s h", h=NH),
                        in0=kg[:, :, 0:8],
                        in1=kg[:, :, 8:16],
                        op=AL.add,
                    )
                    # --- softmax over s (per head); shift-free: exp args are
                    # in [-14, -5], safe in fp32 ---
                    E = L
                    nc.scalar.activation(
                        out=E[:], in_=L[:], func=mybir.ActivationFunctionType.Exp,
                        scale=-SCALE,
                    )
                    den = smp.tile([128, NH], f32, tag="den")
                    nc.vector.tensor_reduce(
                        out=den[:],
                        in_=E[:].rearrange("p (s h) -> p h s", h=NH),
                        axis=mybir.AxisListType.X,
                        op=AL.add,
                    )
                    rden = smp.tile([128, NH], f32, tag="rden")
                    nc.vector.reciprocal(rden[:], den[:])
                    Wt = smp.tile([128, S * NH], bf16, tag="Wt")
                    nc.vector.tensor_tensor(
                        out=Wt[:].rearrange("p (s h) -> p s h", h=NH),
                        in0=E[:].rearrange("p (s h) -> p s h", h=NH),
                        in1=rden[:, None, :].to_broadcast([128, S, NH]),
                        op=AL.mult,
                    )
                    # --- weighted v sum: vg <- vg * Wt, fold s 32 -> 1 ---
                    vgv = vg[:].rearrange("p s (w h) -> p s w h", h=NH)
                    wv = Wt[:].rearrange("p (s h) -> p s h", h=NH)[
                        :, :, None, :
                    ].to_broadcast([128, S, W, NH])
                    nc.vector.tensor_tensor(
                        out=vgv, in0=vgv, in1=wv, op=AL.mult,
                    )
                    for sd in (16, 8, 4, 2, 1):
                        nc.vector.tensor_tensor(
                            out=vg[:, 0:sd, :],
                            in0=vg[:, 0:sd, :],
                            in1=vg[:, sd : 2 * sd, :],
                            op=AL.add,
                        )
                    nc.sync.dma_start(out=oc[b, c], in_=vg[:, 0, :])
    nc.compile()
    return nc


_NC_CACHE = None


def kernel(v, q, k, coo, dst_mxlen):
    global _NC_CACHE
    assert int(dst_mxlen) == S
    v = np.asarray(v, dtype=np.float32)
    q = np.asarray(q, dtype=np.float32)
    k = np.asarray(k, dtype=np.float32)
    coo = np.asarray(coo)

    # src table: srct[t, s] = src index of edge (dst=t, slot=s)
    srct = np.zeros((N_TOK, S), dtype=np.int64)
    srct[coo[:, 0], coo[:, 2]] = coo[:, 1]

    # w-major bf16 tables: row (b, tok) = [w, h] flattened
    kf = np.ascontiguousarray(
        k.transpose(0, 1, 3, 2).reshape(BS * N_TOK, HW)
    ).astype(bfloat16)
    vf = np.ascontiguousarray(
        v.transpose(0, 1, 3, 2).reshape(BS * N_TOK, HW)
    ).astype(bfloat16)

    if _NC_CACHE is None:
        _NC_CACHE = build_kernel()
    nc = _NC_CACHE

    in_maps = []
    for core in range(N_CORES):
        lo = core * DT
        qcore = np.ascontiguousarray(
            q[:, lo : lo + DT].transpose(0, 1, 3, 2).reshape(BS, CHUNKS, 128, HW)
        ).astype(bfloat16)
        idx = np.zeros((BS, CHUNKS, 4, 128, EDGES // 64), dtype=np.int16)
        for b in range(BS):
            for c in range(CHUNKS):
                # edge i = s*128 + p  ->  row b*2048 + srct[lo + c*128 + p, s]
                flat = (
                    b * N_TOK + srct[lo + c * 128 : lo + (c + 1) * 128, :].T
                ).reshape(-1).astype(np.int16)
                for hf in range(4):
                    idx[b, c, hf] = _wrap_idx(
                        flat[EDGES // 4 * hf : EDGES // 4 * (hf + 1)]
                    )
        in_maps.append({"kf": kf, "vf": vf, "qc": qcore, "idx": idx})

    res = run_bass_kernel_spmd(nc, in_maps, list(range(N_CORES)))
    out = np.empty((BS, N_TOK, NH, W), dtype=np.float32)
    for core in range(N_CORES):
        lo = core * DT
        o = np.asarray(res.results[core]["oc"]).astype(np.float32)
        o = o.reshape(BS, CHUNKS, 128, W, NH).transpose(0, 1, 2, 4, 3)
        out[:, lo : lo + DT] = o.reshape(BS, DT, NH, W)
    return out
